# revision 7
# baseline (speedup 1.0000x reference)
"""KAN layer (LayerNorm -> per-bin Bernstein spline -> reduce over input dim)
as a Bass/Tile kernel for 8 trn2 NeuronCores.

Sharding: data-parallel over batch (8 rows of x per core), poly_matrix
replicated, rearranged on the host to R[i*GRID+g, k*D_OUT+o], quantized to
uint8 (u = round(P/s + 127.5), P ~= s*(u - 127.5)), and byte-permuted within
each 2KB row (even bytes = first half, odd bytes = second half).

u8 gather + fp16 halfword unpack: the indirect gather moves uint8 rows
(HALF the bf16 DMA traffic, the dominant cost).  Two fused DVE bitwise ops
per tile then build fp16 halfwords directly:
    lo = (pair & 0x00FF) | 0x5800     hi = (pair >> 8) | 0x5800
0x58 is the high byte of fp16 128.0, so each halfword reads as the fp16
value 128 + u/8 -- exactly linear in the full 8-bit payload -- and the PE
consumes the quantized data with no dequant pass.  The affine offset is
removed exactly via the Bernstein partition of unity (sum_k bv_k = 1):
    y[b,o] = 8*s*MM[b,o] - s*1151.5*SUMBV[b]
where MM = sum_{i,k} bv_fp16[b,i,k] * (128 + u/8) and SUMBV[b] = sum bv_fp16,
computed on device with a ones-vector matmul.

Math per core (b in 0..7):
  ln/bins:  xn = LN(x); u = ((clip(xn)+1)*0.5)*GRID; g = floor(u); t = u-g
  basis:    bv[b,i,k] = Bernstein_k(t) via Horner, rounded to fp16
  gather:   one 2KB u8 row per (b,i) via indirect DMA, 128 rows/instruction
  unpack:   2 DVE tensor_scalar ops per gathered tile
  contract: PE matmuls, lhsT=bv column [128,1] fp16, rhs=Gf k-slice [128,512]
            fp16, accumulated in PSUM over 4 i-chunks x 4 k.
"""

import numpy as np

import concourse.bass as bass
import concourse.mybir as mybir
import concourse.tile as tile
from concourse import bacc
from concourse.bass_utils import run_bass_kernel_spmd
from concourse.masks import make_identity

B = 64          # total batch
D_IN = 512
D_OUT = 512
DEG = 3
GRID = 100
GRID_EPS = 1e-6
LN_EPS = 1e-5
N_CORES = 8
BPC = B // N_CORES          # batch rows per core (8)
NROWS = D_IN * GRID         # 51200 gatherable rows
ROW = (DEG + 1) * D_OUT     # 2048 elements per row (k-major, o-minor)
NCH = D_IN // 128           # 4 i-chunks of 128

F32 = mybir.dt.float32
F16 = mybir.dt.float16
U8 = mybir.dt.uint8
U16 = mybir.dt.uint16
I32 = mybir.dt.int32
AX = mybir.AxisListType
OP = mybir.AluOpType
AF = mybir.ActivationFunctionType

_CACHE = {}
_RPREP = {}


def _build_nc(Mconst, apply_affine, s, repeat=1):
    """Build + compile the per-core Bass program.

    Mconst: 4x4 python floats of basis_matrix (power->Bernstein).
    apply_affine: apply ln_weight/ln_bias tiles (skipped when they are 1/0).
    s: uint8 quantization scale (P ~= s*(u-127.5)).
    repeat: unroll the gather+contract phase this many times (timing builds).
    """
    nc = bacc.Bacc("TRN2", target_bir_lowering=False, debug=False)

    x8 = nc.declare_dram_parameter("x8", [BPC, D_IN], F32, isOutput=False)
    R = nc.declare_dram_parameter("R", [NROWS, ROW], U8, isOutput=False)
    if apply_affine:
        w8 = nc.declare_dram_parameter("w8", [BPC, D_IN], F32, isOutput=False)
        b8 = nc.declare_dram_parameter("b8", [BPC, D_IN], F32, isOutput=False)
    y8 = nc.declare_dram_parameter("y8", [BPC, D_OUT], F32, isOutput=True)

    with tile.TileContext(nc) as tc:
        with (
            tc.tile_pool(name="const", bufs=1) as cp,
            tc.tile_pool(name="work", bufs=1) as wp,
            tc.tile_pool(name="g8pool", bufs=8) as g8p,
            tc.tile_pool(name="gfpool", bufs=8) as gfp,
            tc.tile_pool(name="outp", bufs=2) as op_,
            tc.tile_pool(name="ptr", bufs=2, space="PSUM") as ptr,
            tc.tile_pool(name="pacc", bufs=1, space="PSUM") as pacc,
        ):
            ident = cp.tile([128, 128], F32, tag="ident")
            make_identity(nc, ident[:])

            # row-base offsets i*GRID, as f32 (exact ints < 2^24)
            iotaI = cp.tile([BPC, D_IN], I32, tag="iotaI")
            nc.gpsimd.iota(iotaI[:], pattern=[[GRID, D_IN]], base=0,
                           channel_multiplier=0)
            iotaF = cp.tile([BPC, D_IN], F32, tag="iotaF")
            nc.vector.tensor_copy(iotaF[:], iotaI[:])

            x = wp.tile([BPC, D_IN], F32, tag="x")
            nc.sync.dma_start(x[:], x8[:])

            # ---- LayerNorm (two-pass, matching jnp.mean/jnp.var) ----
            sumx = wp.tile([BPC, 1], F32, tag="sumx")
            nc.vector.tensor_reduce(sumx[:], x[:], axis=AX.X, op=OP.add)
            mean = wp.tile([BPC, 1], F32, tag="mean")
            nc.vector.tensor_scalar_mul(mean[:], sumx[:], 1.0 / D_IN)
            xc = wp.tile([BPC, D_IN], F32, tag="xc")
            nc.vector.tensor_scalar(xc[:], x[:], mean[:, :1], None, OP.subtract)
            sq = wp.tile([BPC, D_IN], F32, tag="sq")
            nc.scalar.square(sq[:], xc[:])
            v = wp.tile([BPC, 1], F32, tag="v")
            nc.vector.tensor_reduce(v[:], sq[:], axis=AX.X, op=OP.add)
            # v = sumsq/D + eps
            nc.vector.tensor_scalar(v[:], v[:], 1.0 / D_IN, LN_EPS, OP.mult, OP.add)
            # rstd = rsqrt(v) via sqrt + reciprocal + one Newton step
            s_ = wp.tile([BPC, 1], F32, tag="s")
            nc.scalar.sqrt(s_[:], v[:])
            r0 = wp.tile([BPC, 1], F32, tag="r0")
            nc.vector.reciprocal(r0[:], s_[:])
            r2 = wp.tile([BPC, 1], F32, tag="r2")
            nc.vector.tensor_tensor(out=r2[:], in0=r0[:], in1=r0[:], op=OP.mult)
            nc.vector.tensor_tensor(out=r2[:], in0=r2[:], in1=v[:], op=OP.mult)
            nc.vector.tensor_scalar(r2[:], r2[:], -0.5, 1.5, OP.mult, OP.add)
            rstd = wp.tile([BPC, 1], F32, tag="rstd")
            nc.vector.tensor_tensor(out=rstd[:], in0=r0[:], in1=r2[:], op=OP.mult)

            xn = wp.tile([BPC, D_IN], F32, tag="xn")
            nc.vector.tensor_scalar(xn[:], xc[:], rstd[:, :1], None, OP.mult)
            if apply_affine:
                wt = wp.tile([BPC, D_IN], F32, tag="wt")
                bt = wp.tile([BPC, D_IN], F32, tag="bt")
                nc.sync.dma_start(wt[:], w8[:])
                nc.sync.dma_start(bt[:], b8[:])
                nc.vector.tensor_tensor(out=xn[:], in0=xn[:], in1=wt[:], op=OP.mult)
                nc.vector.tensor_tensor(out=xn[:], in0=xn[:], in1=bt[:], op=OP.add)

            # clip, map to [0, GRID)
            cl = wp.tile([BPC, D_IN], F32, tag="cl")
            nc.vector.tensor_scalar(cl[:], xn[:], -1.0 + GRID_EPS, 1.0 - GRID_EPS,
                                    OP.max, OP.min)
            # ((cl + 1) * 0.5) * 100  -- same op order as the reference
            u = wp.tile([BPC, D_IN], F32, tag="u")
            nc.vector.tensor_scalar(u[:], cl[:], 1.0, 0.5, OP.add, OP.mult)
            nc.vector.tensor_scalar(u[:], u[:], float(GRID), None, OP.mult)

            # floor(u) robust to either int-conversion rounding mode
            i1 = wp.tile([BPC, D_IN], I32, tag="i1")
            nc.vector.tensor_copy(i1[:], u[:])
            f1 = wp.tile([BPC, D_IN], F32, tag="f1")
            nc.vector.tensor_copy(f1[:], i1[:])
            gt = wp.tile([BPC, D_IN], F32, tag="gt")
            nc.vector.tensor_tensor(out=gt[:], in0=f1[:], in1=u[:], op=OP.is_gt)
            flr = wp.tile([BPC, D_IN], F32, tag="flr")
            nc.vector.tensor_tensor(out=flr[:], in0=f1[:], in1=gt[:], op=OP.subtract)
            t = wp.tile([BPC, D_IN], F32, tag="t")
            nc.vector.tensor_tensor(out=t[:], in0=u[:], in1=flr[:], op=OP.subtract)
            offsF = wp.tile([BPC, D_IN], F32, tag="offsF")
            nc.vector.tensor_tensor(out=offsF[:], in0=flr[:], in1=iotaF[:], op=OP.add)

            # ---- Bernstein basis via Horner ----
            bv = []
            for k in range(DEG + 1):
                m3, m2, m1, m0 = (Mconst[3][k], Mconst[2][k],
                                  Mconst[1][k], Mconst[0][k])
                h = wp.tile([BPC, D_IN], F32, tag=f"bv{k}")
                nc.scalar.activation(h[:], t[:], AF.Copy, bias=m2, scale=m3)
                nc.vector.tensor_tensor(out=h[:], in0=h[:], in1=t[:], op=OP.mult)
                nc.scalar.activation(h[:], h[:], AF.Copy, bias=m1, scale=1.0)
                nc.vector.tensor_tensor(out=h[:], in0=h[:], in1=t[:], op=OP.mult)
                nc.scalar.activation(h[:], h[:], AF.Copy, bias=m0, scale=1.0)
                bv.append(h)

            # ---- transpose offsets and bv to [128 i, BPC b] ----
            offsT = []
            bvT = [[None] * NCH for _ in range(DEG + 1)]
            for c in range(NCH):
                sl = slice(c * 128, (c + 1) * 128)
                pt = ptr.tile([128, BPC], F32, tag="ptr")
                nc.tensor.transpose(pt[:], offsF[:, sl], ident[:BPC, :BPC])
                ot = cp.tile([128, BPC], I32, tag=f"offsT{c}")
                nc.vector.tensor_copy(ot[:], pt[:])
                offsT.append(ot)
                for k in range(DEG + 1):
                    pb = ptr.tile([128, BPC], F32, tag="ptr")
                    nc.tensor.transpose(pb[:], bv[k][:, sl], ident[:BPC, :BPC])
                    bt_ = cp.tile([128, BPC], F16, tag=f"bvT{k}_{c}")
                    nc.vector.tensor_copy(bt_[:], pb[:])
                    bvT[k][c] = bt_

            # ---- SUMBV[b] = sum_{i,k} fp16(bv) via ones-vector matmuls ----
            ones = cp.tile([128, 1], F16, tag="ones")
            nc.vector.memset(ones[:], 1.0)
            sacc = pacc.tile([1, BPC], F32, tag="sacc")
            n_mm = NCH * (DEG + 1)
            j = 0
            for c in range(NCH):
                for k in range(DEG + 1):
                    nc.tensor.matmul(sacc[:], lhsT=ones[:], rhs=bvT[k][c][:],
                                     start=(j == 0), stop=(j == n_mm - 1))
                    j += 1
            # CB[b] = (1151.5/8) * SUMBV[b], row vector on partition 0
            cb = cp.tile([1, BPC], F32, tag="cb")
            nc.vector.tensor_scalar_mul(cb[:], sacc[:], 1151.5 / 8.0)

            # ---- gather + unpack + contract ----
            if repeat == 0:
                zrow = op_.tile([BPC, D_OUT], F32, tag="zrow")
                nc.vector.memset(zrow[:], 0.0)
                nc.sync.dma_start(y8[:, :], zrow[:])
            for _rep in range(repeat):
                for b in range(BPC):
                    acc = pacc.tile([1, D_OUT], F32, tag="acc", bufs=4,
                                    name="acc")
                    for c in range(NCH):
                        G8 = g8p.tile([128, ROW], U8, tag="G8", name="G8")
                        nc.gpsimd.indirect_dma_start(
                            out=G8[:],
                            out_offset=None,
                            in_=R[:],
                            in_offset=bass.IndirectOffsetOnAxis(
                                ap=offsT[c][:, b:b + 1], axis=0),
                        )
                        Gf = gfp.tile([128, ROW], F16, tag="Gf", name="Gf")
                        x16 = G8[:].bitcast(U16)
                        nc.vector.tensor_scalar(
                            Gf[:, 0:ROW // 2].bitcast(U16), x16,
                            0x00FF, 0x5800, OP.bitwise_and, OP.bitwise_or)
                        nc.vector.tensor_scalar(
                            Gf[:, ROW // 2:ROW].bitcast(U16), x16,
                            8, 0x5800, OP.logical_shift_right, OP.bitwise_or)
                        for k in range(DEG + 1):
                            nc.tensor.matmul(
                                acc[:],
                                lhsT=bvT[k][c][:, b:b + 1],
                                rhs=Gf[:, k * D_OUT:(k + 1) * D_OUT],
                                start=(c == 0 and k == 0),
                                stop=(c == NCH - 1 and k == DEG),
                            )
                    orow = op_.tile([1, D_OUT], F32, tag="orow")
                    nc.vector.tensor_scalar(orow[:], acc[:],
                                            cb[0:1, b:b + 1], 8.0 * s,
                                            OP.subtract, OP.mult)
                    nc.sync.dma_start(y8[b:b + 1, :], orow[:])

    nc.compile()
    return nc


def prep_R(poly_matrix):
    """poly[i, o, g, k] -> R[i, g, k, o] -> [NROWS, ROW] uint8, with each
    row byte-permuted so the fp16 unpack lands contiguously:
    stored[2m] = u[m], stored[2m+1] = u[1024+m].

    Returns (R_u8, s) with P ~= s * (u - 127.5)."""
    key = poly_matrix.ctypes.data if isinstance(poly_matrix, np.ndarray) else None
    if key is not None and key in _RPREP:
        return _RPREP[key]
    Rm = np.ascontiguousarray(np.transpose(np.asarray(poly_matrix, np.float32),
                                           (0, 2, 3, 1))).reshape(NROWS, ROW)
    s = float(np.abs(Rm).max()) / 127.5
    u = np.clip(np.round(Rm / s + 127.5), 0, 255).astype(np.uint8)
    perm = np.empty_like(u)
    perm[:, 0::2] = u[:, :ROW // 2]
    perm[:, 1::2] = u[:, ROW // 2:]
    out = (np.ascontiguousarray(perm), s)
    if key is not None:
        _RPREP[key] = out
    return out


def get_compiled(basis_matrix, ln_weight, ln_bias, s, repeat=1):
    apply_affine = not (np.all(ln_weight == 1.0) and np.all(ln_bias == 0.0))
    Mkey = np.asarray(basis_matrix, np.float32).tobytes()
    key = (Mkey, apply_affine, s, repeat)
    if key not in _CACHE:
        Mconst = [[float(basis_matrix[j, k]) for k in range(DEG + 1)]
                  for j in range(DEG + 1)]
        _CACHE[key] = _build_nc(Mconst, apply_affine, s, repeat)
    return _CACHE[key], apply_affine


def make_in_maps(x, R_u8, ln_weight, ln_bias, apply_affine):
    x = np.asarray(x, np.float32)
    maps = []
    for c in range(N_CORES):
        m = {"x8": np.ascontiguousarray(x[c * BPC:(c + 1) * BPC]), "R": R_u8}
        if apply_affine:
            m["w8"] = np.ascontiguousarray(
                np.broadcast_to(np.asarray(ln_weight, np.float32), (BPC, D_IN)))
            m["b8"] = np.ascontiguousarray(
                np.broadcast_to(np.asarray(ln_bias, np.float32), (BPC, D_IN)))
        maps.append(m)
    return maps


def kernel(x, poly_matrix, ln_weight, ln_bias, basis_matrix):
    R_u8, s = prep_R(np.asarray(poly_matrix))
    nc, apply_affine = get_compiled(basis_matrix, ln_weight, ln_bias, s)
    in_maps = make_in_maps(x, R_u8, ln_weight, ln_bias, apply_affine)
    res = run_bass_kernel_spmd(nc, in_maps, core_ids=list(range(N_CORES)))
    y = np.concatenate([res.results[c]["y8"] for c in range(N_CORES)], axis=0)
    return y.astype(np.float32)


# revision 9
# speedup vs baseline: 1.0029x; 1.0029x over previous
"""KAN layer (LayerNorm -> per-bin Bernstein spline -> reduce over input dim)
as a Bass/Tile kernel for 8 trn2 NeuronCores.

Sharding: data-parallel over batch (8 rows of x per core), poly_matrix
replicated, rearranged on the host to R[i*GRID+g, k*D_OUT+o], quantized to
uint8 (u = round(P/s + 127.5), P ~= s*(u - 127.5)), and byte-permuted within
each 2KB row (even bytes = first half, odd bytes = second half).

u8 gather + fp16 halfword unpack: the indirect gather moves uint8 rows
(HALF the bf16 DMA traffic, the dominant cost).  Two fused DVE bitwise ops
per tile then build fp16 halfwords directly:
    lo = (pair & 0x00FF) | 0x5800     hi = (pair >> 8) | 0x5800
0x58 is the high byte of fp16 128.0, so each halfword reads as the fp16
value 128 + u/8 -- exactly linear in the full 8-bit payload -- and the PE
consumes the quantized data with no dequant pass.  The affine offset is
removed exactly via the Bernstein partition of unity (sum_k bv_k = 1):
    y[b,o] = 8*s*MM[b,o] - s*1151.5*SUMBV[b]
where MM = sum_{i,k} bv_fp16[b,i,k] * (128 + u/8) and SUMBV[b] = sum bv_fp16,
computed on device with a ones-vector matmul.

Math per core (b in 0..7):
  ln/bins:  xn = LN(x); u = ((clip(xn)+1)*0.5)*GRID; g = floor(u); t = u-g
  basis:    bv[b,i,k] = Bernstein_k(t) via Horner, rounded to fp16
  gather:   one 2KB u8 row per (b,i) via indirect DMA, 128 rows/instruction
  unpack:   2 DVE tensor_scalar ops per gathered tile
  contract: PE matmuls, lhsT=bv column [128,1] fp16, rhs=Gf k-slice [128,512]
            fp16, accumulated in PSUM over 4 i-chunks x 4 k.
"""

import numpy as np

import concourse.bass as bass
import concourse.mybir as mybir
import concourse.tile as tile
from concourse import bacc
from concourse.bass_utils import run_bass_kernel_spmd
from concourse.masks import make_identity

B = 64          # total batch
D_IN = 512
D_OUT = 512
DEG = 3
GRID = 100
GRID_EPS = 1e-6
LN_EPS = 1e-5
N_CORES = 8
BPC = B // N_CORES          # batch rows per core (8)
NROWS = D_IN * GRID         # 51200 gatherable rows
ROW = (DEG + 1) * D_OUT     # 2048 elements per row (k-major, o-minor)
NCH = D_IN // 128           # 4 i-chunks of 128

F32 = mybir.dt.float32
F16 = mybir.dt.float16
U8 = mybir.dt.uint8
U16 = mybir.dt.uint16
I32 = mybir.dt.int32
AX = mybir.AxisListType
OP = mybir.AluOpType
AF = mybir.ActivationFunctionType

_CACHE = {}
_RPREP = {}


def _build_nc(Mconst, apply_affine, s, repeat=1):
    """Build + compile the per-core Bass program.

    Mconst: 4x4 python floats of basis_matrix (power->Bernstein).
    apply_affine: apply ln_weight/ln_bias tiles (skipped when they are 1/0).
    s: uint8 quantization scale (P ~= s*(u-127.5)).
    repeat: unroll the gather+contract phase this many times (timing builds).
    """
    nc = bacc.Bacc("TRN2", target_bir_lowering=False, debug=False)

    x8 = nc.declare_dram_parameter("x8", [BPC, D_IN], F32, isOutput=False)
    R = nc.declare_dram_parameter("R", [NROWS, ROW], U8, isOutput=False)
    if apply_affine:
        w8 = nc.declare_dram_parameter("w8", [BPC, D_IN], F32, isOutput=False)
        b8 = nc.declare_dram_parameter("b8", [BPC, D_IN], F32, isOutput=False)
    y8 = nc.declare_dram_parameter("y8", [BPC, D_OUT], F32, isOutput=True)

    with tile.TileContext(nc) as tc:
        with (
            tc.tile_pool(name="const", bufs=1) as cp,
            tc.tile_pool(name="work", bufs=1) as wp,
            tc.tile_pool(name="g8pool", bufs=8) as g8p,
            tc.tile_pool(name="gfpool", bufs=8) as gfp,
            tc.tile_pool(name="outp", bufs=2) as op_,
            tc.tile_pool(name="ptr", bufs=1, space="PSUM") as ptr,
            tc.tile_pool(name="pacc", bufs=1, space="PSUM") as pacc,
        ):
            ident = cp.tile([128, 128], F32, tag="ident")
            make_identity(nc, ident[:])

            # row-base offsets i*GRID, as f32 (exact ints < 2^24)
            iotaI = cp.tile([BPC, D_IN], I32, tag="iotaI")
            nc.gpsimd.iota(iotaI[:], pattern=[[GRID, D_IN]], base=0,
                           channel_multiplier=0)
            iotaF = cp.tile([BPC, D_IN], F32, tag="iotaF")
            nc.vector.tensor_copy(iotaF[:], iotaI[:])

            x = wp.tile([BPC, D_IN], F32, tag="x")
            nc.sync.dma_start(x[:], x8[:])

            # ---- LayerNorm (two-pass, matching jnp.mean/jnp.var) ----
            sumx = wp.tile([BPC, 1], F32, tag="sumx")
            nc.vector.tensor_reduce(sumx[:], x[:], axis=AX.X, op=OP.add)
            mean = wp.tile([BPC, 1], F32, tag="mean")
            nc.vector.tensor_scalar_mul(mean[:], sumx[:], 1.0 / D_IN)
            xc = wp.tile([BPC, D_IN], F32, tag="xc")
            nc.vector.tensor_scalar(xc[:], x[:], mean[:, :1], None, OP.subtract)
            sq = wp.tile([BPC, D_IN], F32, tag="sq")
            nc.scalar.square(sq[:], xc[:])
            v = wp.tile([BPC, 1], F32, tag="v")
            nc.vector.tensor_reduce(v[:], sq[:], axis=AX.X, op=OP.add)
            # v = sumsq/D + eps
            nc.vector.tensor_scalar(v[:], v[:], 1.0 / D_IN, LN_EPS, OP.mult, OP.add)
            # rstd = rsqrt(v) via sqrt + reciprocal + one Newton step
            s_ = wp.tile([BPC, 1], F32, tag="s")
            nc.scalar.sqrt(s_[:], v[:])
            r0 = wp.tile([BPC, 1], F32, tag="r0")
            nc.vector.reciprocal(r0[:], s_[:])
            r2 = wp.tile([BPC, 1], F32, tag="r2")
            nc.vector.tensor_tensor(out=r2[:], in0=r0[:], in1=r0[:], op=OP.mult)
            nc.vector.tensor_tensor(out=r2[:], in0=r2[:], in1=v[:], op=OP.mult)
            nc.vector.tensor_scalar(r2[:], r2[:], -0.5, 1.5, OP.mult, OP.add)
            rstd = wp.tile([BPC, 1], F32, tag="rstd")
            nc.vector.tensor_tensor(out=rstd[:], in0=r0[:], in1=r2[:], op=OP.mult)

            xn = wp.tile([BPC, D_IN], F32, tag="xn")
            nc.vector.tensor_scalar(xn[:], xc[:], rstd[:, :1], None, OP.mult)
            if apply_affine:
                wt = wp.tile([BPC, D_IN], F32, tag="wt")
                bt = wp.tile([BPC, D_IN], F32, tag="bt")
                nc.sync.dma_start(wt[:], w8[:])
                nc.sync.dma_start(bt[:], b8[:])
                nc.vector.tensor_tensor(out=xn[:], in0=xn[:], in1=wt[:], op=OP.mult)
                nc.vector.tensor_tensor(out=xn[:], in0=xn[:], in1=bt[:], op=OP.add)

            # clip, map to [0, GRID)
            cl = wp.tile([BPC, D_IN], F32, tag="cl")
            nc.vector.tensor_scalar(cl[:], xn[:], -1.0 + GRID_EPS, 1.0 - GRID_EPS,
                                    OP.max, OP.min)
            # ((cl + 1) * 0.5) * 100  -- same op order as the reference
            u = wp.tile([BPC, D_IN], F32, tag="u")
            nc.vector.tensor_scalar(u[:], cl[:], 1.0, 0.5, OP.add, OP.mult)
            nc.vector.tensor_scalar(u[:], u[:], float(GRID), None, OP.mult)

            # floor(u) robust to either int-conversion rounding mode
            i1 = wp.tile([BPC, D_IN], I32, tag="i1")
            nc.vector.tensor_copy(i1[:], u[:])
            f1 = wp.tile([BPC, D_IN], F32, tag="f1")
            nc.vector.tensor_copy(f1[:], i1[:])
            gt = wp.tile([BPC, D_IN], F32, tag="gt")
            nc.vector.tensor_tensor(out=gt[:], in0=f1[:], in1=u[:], op=OP.is_gt)
            flr = wp.tile([BPC, D_IN], F32, tag="flr")
            nc.vector.tensor_tensor(out=flr[:], in0=f1[:], in1=gt[:], op=OP.subtract)
            t = wp.tile([BPC, D_IN], F32, tag="t")
            nc.vector.tensor_tensor(out=t[:], in0=u[:], in1=flr[:], op=OP.subtract)
            offsF = wp.tile([BPC, D_IN], F32, tag="offsF")
            nc.vector.tensor_tensor(out=offsF[:], in0=flr[:], in1=iotaF[:], op=OP.add)

            # ---- Bernstein basis via Horner ----
            bv = []
            for k in range(DEG + 1):
                m3, m2, m1, m0 = (Mconst[3][k], Mconst[2][k],
                                  Mconst[1][k], Mconst[0][k])
                h = wp.tile([BPC, D_IN], F32, tag=f"bv{k}")
                nc.scalar.activation(h[:], t[:], AF.Copy, bias=m2, scale=m3)
                nc.vector.tensor_tensor(out=h[:], in0=h[:], in1=t[:], op=OP.mult)
                nc.scalar.activation(h[:], h[:], AF.Copy, bias=m1, scale=1.0)
                nc.vector.tensor_tensor(out=h[:], in0=h[:], in1=t[:], op=OP.mult)
                nc.scalar.activation(h[:], h[:], AF.Copy, bias=m0, scale=1.0)
                bv.append(h)

            # ---- transpose offsets and bv to [128 i, BPC b] ----
            offsT = []
            bvT = [[None] * NCH for _ in range(DEG + 1)]
            for c in range(NCH):
                sl = slice(c * 128, (c + 1) * 128)
                pt = ptr.tile([128, BPC], F32, tag="ptr")
                nc.tensor.transpose(pt[:], offsF[:, sl], ident[:BPC, :BPC])
                ot = cp.tile([128, BPC], I32, tag=f"offsT{c}")
                nc.vector.tensor_copy(ot[:], pt[:])
                offsT.append(ot)
                for k in range(DEG + 1):
                    pb = ptr.tile([128, BPC], F32, tag="ptr")
                    nc.tensor.transpose(pb[:], bv[k][:, sl], ident[:BPC, :BPC])
                    bt_ = cp.tile([128, BPC], F16, tag=f"bvT{k}_{c}")
                    nc.vector.tensor_copy(bt_[:], pb[:])
                    bvT[k][c] = bt_

            # ---- SUMBV[b] = sum_{i,k} fp16(bv) via ones-vector matmuls ----
            ones = cp.tile([128, 1], F16, tag="ones")
            nc.vector.memset(ones[:], 1.0)
            sacc = ptr.tile([1, BPC], F32, tag="ptr", name="sacc")
            n_mm = NCH * (DEG + 1)
            j = 0
            for c in range(NCH):
                for k in range(DEG + 1):
                    nc.tensor.matmul(sacc[:], lhsT=ones[:], rhs=bvT[k][c][:],
                                     start=(j == 0), stop=(j == n_mm - 1))
                    j += 1
            # CB[b] = (1151.5/8) * SUMBV[b], row vector on partition 0
            cb = cp.tile([1, BPC], F32, tag="cb")
            nc.vector.tensor_scalar_mul(cb[:], sacc[:], 1151.5 / 8.0)

            # ---- gather + unpack + contract ----
            if repeat == 0:
                zrow = op_.tile([BPC, D_OUT], F32, tag="zrow")
                nc.vector.memset(zrow[:], 0.0)
                nc.sync.dma_start(y8[:, :], zrow[:])
            for _rep in range(repeat):
                accs = [pacc.tile([1, D_OUT], F32, tag=f"acc{b}",
                                  name=f"acc{b}")
                        for b in range(BPC - 1)]
                accs.append(ptr.tile([1, D_OUT], F32, tag="ptr", name="acc7"))
                for b in range(BPC):
                    acc = accs[b]
                    for c in range(NCH):
                        G8 = g8p.tile([128, ROW], U8, tag="G8", name="G8")
                        nc.gpsimd.indirect_dma_start(
                            out=G8[:],
                            out_offset=None,
                            in_=R[:],
                            in_offset=bass.IndirectOffsetOnAxis(
                                ap=offsT[c][:, b:b + 1], axis=0),
                        )
                        Gf = gfp.tile([128, ROW], F16, tag="Gf", name="Gf")
                        x16 = G8[:].bitcast(U16)
                        nc.vector.tensor_scalar(
                            Gf[:, 0:ROW // 2].bitcast(U16), x16,
                            0x00FF, 0x5800, OP.bitwise_and, OP.bitwise_or)
                        nc.vector.tensor_scalar(
                            Gf[:, ROW // 2:ROW].bitcast(U16), x16,
                            8, 0x5800, OP.logical_shift_right, OP.bitwise_or)
                        for k in range(DEG + 1):
                            nc.tensor.matmul(
                                acc[:],
                                lhsT=bvT[k][c][:, b:b + 1],
                                rhs=Gf[:, k * D_OUT:(k + 1) * D_OUT],
                                start=(c == 0 and k == 0),
                                stop=(c == NCH - 1 and k == DEG),
                            )
                # all epilogues after the full matmul stream: the in-order
                # DVE must not stall mid-iteration behind PE-dependent work
                for b in range(BPC):
                    orow = op_.tile([1, D_OUT], F32, tag="orow")
                    nc.vector.tensor_scalar(orow[:], accs[b][:],
                                            cb[0:1, b:b + 1], 8.0 * s,
                                            OP.subtract, OP.mult)
                    nc.sync.dma_start(y8[b:b + 1, :], orow[:])

    nc.compile()
    return nc


def prep_R(poly_matrix):
    """poly[i, o, g, k] -> R[i, g, k, o] -> [NROWS, ROW] uint8, with each
    row byte-permuted so the fp16 unpack lands contiguously:
    stored[2m] = u[m], stored[2m+1] = u[1024+m].

    Returns (R_u8, s) with P ~= s * (u - 127.5)."""
    key = poly_matrix.ctypes.data if isinstance(poly_matrix, np.ndarray) else None
    if key is not None and key in _RPREP:
        return _RPREP[key]
    Rm = np.ascontiguousarray(np.transpose(np.asarray(poly_matrix, np.float32),
                                           (0, 2, 3, 1))).reshape(NROWS, ROW)
    s = float(np.abs(Rm).max()) / 127.5
    u = np.clip(np.round(Rm / s + 127.5), 0, 255).astype(np.uint8)
    perm = np.empty_like(u)
    perm[:, 0::2] = u[:, :ROW // 2]
    perm[:, 1::2] = u[:, ROW // 2:]
    out = (np.ascontiguousarray(perm), s)
    if key is not None:
        _RPREP[key] = out
    return out


def get_compiled(basis_matrix, ln_weight, ln_bias, s, repeat=1):
    apply_affine = not (np.all(ln_weight == 1.0) and np.all(ln_bias == 0.0))
    Mkey = np.asarray(basis_matrix, np.float32).tobytes()
    key = (Mkey, apply_affine, s, repeat)
    if key not in _CACHE:
        Mconst = [[float(basis_matrix[j, k]) for k in range(DEG + 1)]
                  for j in range(DEG + 1)]
        _CACHE[key] = _build_nc(Mconst, apply_affine, s, repeat)
    return _CACHE[key], apply_affine


def make_in_maps(x, R_u8, ln_weight, ln_bias, apply_affine):
    x = np.asarray(x, np.float32)
    maps = []
    for c in range(N_CORES):
        m = {"x8": np.ascontiguousarray(x[c * BPC:(c + 1) * BPC]), "R": R_u8}
        if apply_affine:
            m["w8"] = np.ascontiguousarray(
                np.broadcast_to(np.asarray(ln_weight, np.float32), (BPC, D_IN)))
            m["b8"] = np.ascontiguousarray(
                np.broadcast_to(np.asarray(ln_bias, np.float32), (BPC, D_IN)))
        maps.append(m)
    return maps


def kernel(x, poly_matrix, ln_weight, ln_bias, basis_matrix):
    R_u8, s = prep_R(np.asarray(poly_matrix))
    nc, apply_affine = get_compiled(basis_matrix, ln_weight, ln_bias, s)
    in_maps = make_in_maps(x, R_u8, ln_weight, ln_bias, apply_affine)
    res = run_bass_kernel_spmd(nc, in_maps, core_ids=list(range(N_CORES)))
    y = np.concatenate([res.results[c]["y8"] for c in range(N_CORES)], axis=0)
    return y.astype(np.float32)


# revision 10
# speedup vs baseline: 1.1713x; 1.1679x over previous
"""KAN layer (LayerNorm -> per-bin Bernstein spline -> reduce over input dim)
as a Bass/Tile kernel for 8 trn2 NeuronCores.

Sharding: data-parallel over batch (8 rows of x per core), poly_matrix
replicated, rearranged on the host to R[i*GRID+g, k*D_OUT+o], quantized to
uint8 (u = round(P/s + 127.5), P ~= s*(u - 127.5)), and byte-permuted within
each 2KB row (even bytes = first half, odd bytes = second half).

u8 gather + fp16 halfword unpack: the indirect gather moves uint8 rows
(HALF the bf16 DMA traffic, the dominant cost).  Two fused DVE bitwise ops
per tile then build fp16 halfwords directly:
    lo = (pair & 0x00FF) | 0x5800     hi = (pair >> 8) | 0x5800
0x58 is the high byte of fp16 128.0, so each halfword reads as the fp16
value 128 + u/8 -- exactly linear in the full 8-bit payload -- and the PE
consumes the quantized data with no dequant pass.  The affine offset is
removed exactly via the Bernstein partition of unity (sum_k bv_k = 1):
    y[b,o] = 8*s*MM[b,o] - s*1151.5*SUMBV[b]
where MM = sum_{i,k} bv_fp16[b,i,k] * (128 + u/8) and SUMBV[b] = sum bv_fp16,
computed on device with a ones-vector matmul.

Math per core (b in 0..7):
  ln/bins:  xn = LN(x); u = ((clip(xn)+1)*0.5)*GRID; g = floor(u); t = u-g
  basis:    bv[b,i,k] = Bernstein_k(t) via Horner, rounded to fp16
  gather:   one 2KB u8 row per (b,i) via indirect DMA, 128 rows/instruction
  unpack:   2 DVE tensor_scalar ops per gathered tile
  contract: PE matmuls, lhsT=bv column [128,1] fp16, rhs=Gf k-slice [128,512]
            fp16, accumulated in PSUM over 4 i-chunks x 4 k.
"""

import numpy as np

import concourse.bass as bass
import concourse.mybir as mybir
import concourse.tile as tile
from concourse import bacc
from concourse.bass_utils import run_bass_kernel_spmd
from concourse.masks import make_identity

B = 64          # total batch
D_IN = 512
D_OUT = 512
DEG = 3
GRID = 100
GRID_EPS = 1e-6
LN_EPS = 1e-5
N_CORES = 8
BPC = B // N_CORES          # batch rows per core (8)
NROWS = D_IN * GRID         # 51200 gatherable rows
ROW = (DEG + 1) * D_OUT     # 2048 elements per row (k-major, o-minor)
NCH = D_IN // 128           # 4 i-chunks of 128

F32 = mybir.dt.float32
F16 = mybir.dt.float16
U8 = mybir.dt.uint8
U16 = mybir.dt.uint16
I32 = mybir.dt.int32
AX = mybir.AxisListType
OP = mybir.AluOpType
AF = mybir.ActivationFunctionType

_CACHE = {}
_RPREP = {}


def _build_nc(Mconst, apply_affine, s, repeat=1):
    """Build + compile the per-core Bass program.

    Mconst: 4x4 python floats of basis_matrix (power->Bernstein).
    apply_affine: apply ln_weight/ln_bias tiles (skipped when they are 1/0).
    s: uint8 quantization scale (P ~= s*(u-127.5)).
    repeat: unroll the gather+contract phase this many times (timing builds).
    """
    nc = bacc.Bacc("TRN2", target_bir_lowering=False, debug=False)

    x8 = nc.declare_dram_parameter("x8", [BPC, D_IN], F32, isOutput=False)
    R = nc.declare_dram_parameter("R", [NROWS, ROW], U8, isOutput=False)
    if apply_affine:
        w8 = nc.declare_dram_parameter("w8", [BPC, D_IN], F32, isOutput=False)
        b8 = nc.declare_dram_parameter("b8", [BPC, D_IN], F32, isOutput=False)
    y8 = nc.declare_dram_parameter("y8", [BPC, D_OUT], F32, isOutput=True)

    with tile.TileContext(nc) as tc:
        with (
            tc.tile_pool(name="const", bufs=1) as cp,
            tc.tile_pool(name="work", bufs=1) as wp,
            tc.tile_pool(name="g8pool", bufs=16) as g8p,
            tc.tile_pool(name="gfpool", bufs=12) as gfp,
            tc.tile_pool(name="outp", bufs=2) as op_,
            tc.tile_pool(name="ptr", bufs=1, space="PSUM") as ptr,
            tc.tile_pool(name="pacc", bufs=1, space="PSUM") as pacc,
        ):
            ident = cp.tile([128, 128], F32, tag="ident")
            make_identity(nc, ident[:])

            # row-base offsets i*GRID, as f32 (exact ints < 2^24)
            iotaI = cp.tile([BPC, D_IN], I32, tag="iotaI")
            nc.gpsimd.iota(iotaI[:], pattern=[[GRID, D_IN]], base=0,
                           channel_multiplier=0)
            iotaF = cp.tile([BPC, D_IN], F32, tag="iotaF")
            nc.vector.tensor_copy(iotaF[:], iotaI[:])

            x = wp.tile([BPC, D_IN], F32, tag="x")
            nc.sync.dma_start(x[:], x8[:])

            # ---- LayerNorm (two-pass, matching jnp.mean/jnp.var) ----
            sumx = wp.tile([BPC, 1], F32, tag="sumx")
            nc.vector.tensor_reduce(sumx[:], x[:], axis=AX.X, op=OP.add)
            mean = wp.tile([BPC, 1], F32, tag="mean")
            nc.vector.tensor_scalar_mul(mean[:], sumx[:], 1.0 / D_IN)
            xc = wp.tile([BPC, D_IN], F32, tag="xc")
            nc.vector.tensor_scalar(xc[:], x[:], mean[:, :1], None, OP.subtract)
            sq = wp.tile([BPC, D_IN], F32, tag="sq")
            nc.scalar.square(sq[:], xc[:])
            v = wp.tile([BPC, 1], F32, tag="v")
            nc.vector.tensor_reduce(v[:], sq[:], axis=AX.X, op=OP.add)
            # v = sumsq/D + eps
            nc.vector.tensor_scalar(v[:], v[:], 1.0 / D_IN, LN_EPS, OP.mult, OP.add)
            # rstd = rsqrt(v) via sqrt + reciprocal + one Newton step
            s_ = wp.tile([BPC, 1], F32, tag="s")
            nc.scalar.sqrt(s_[:], v[:])
            r0 = wp.tile([BPC, 1], F32, tag="r0")
            nc.vector.reciprocal(r0[:], s_[:])
            r2 = wp.tile([BPC, 1], F32, tag="r2")
            nc.vector.tensor_tensor(out=r2[:], in0=r0[:], in1=r0[:], op=OP.mult)
            nc.vector.tensor_tensor(out=r2[:], in0=r2[:], in1=v[:], op=OP.mult)
            nc.vector.tensor_scalar(r2[:], r2[:], -0.5, 1.5, OP.mult, OP.add)
            rstd = wp.tile([BPC, 1], F32, tag="rstd")
            nc.vector.tensor_tensor(out=rstd[:], in0=r0[:], in1=r2[:], op=OP.mult)

            xn = wp.tile([BPC, D_IN], F32, tag="xn")
            nc.vector.tensor_scalar(xn[:], xc[:], rstd[:, :1], None, OP.mult)
            if apply_affine:
                wt = wp.tile([BPC, D_IN], F32, tag="wt")
                bt = wp.tile([BPC, D_IN], F32, tag="bt")
                nc.sync.dma_start(wt[:], w8[:])
                nc.sync.dma_start(bt[:], b8[:])
                nc.vector.tensor_tensor(out=xn[:], in0=xn[:], in1=wt[:], op=OP.mult)
                nc.vector.tensor_tensor(out=xn[:], in0=xn[:], in1=bt[:], op=OP.add)

            # clip, map to [0, GRID)
            cl = wp.tile([BPC, D_IN], F32, tag="cl")
            nc.vector.tensor_scalar(cl[:], xn[:], -1.0 + GRID_EPS, 1.0 - GRID_EPS,
                                    OP.max, OP.min)
            # ((cl + 1) * 0.5) * 100  -- same op order as the reference
            u = wp.tile([BPC, D_IN], F32, tag="u")
            nc.vector.tensor_scalar(u[:], cl[:], 1.0, 0.5, OP.add, OP.mult)
            nc.vector.tensor_scalar(u[:], u[:], float(GRID), None, OP.mult)

            # floor(u) robust to either int-conversion rounding mode
            i1 = wp.tile([BPC, D_IN], I32, tag="i1")
            nc.vector.tensor_copy(i1[:], u[:])
            f1 = wp.tile([BPC, D_IN], F32, tag="f1")
            nc.vector.tensor_copy(f1[:], i1[:])
            gt = wp.tile([BPC, D_IN], F32, tag="gt")
            nc.vector.tensor_tensor(out=gt[:], in0=f1[:], in1=u[:], op=OP.is_gt)
            flr = wp.tile([BPC, D_IN], F32, tag="flr")
            nc.vector.tensor_tensor(out=flr[:], in0=f1[:], in1=gt[:], op=OP.subtract)
            t = wp.tile([BPC, D_IN], F32, tag="t")
            nc.vector.tensor_tensor(out=t[:], in0=u[:], in1=flr[:], op=OP.subtract)
            offsF = wp.tile([BPC, D_IN], F32, tag="offsF")
            nc.vector.tensor_tensor(out=offsF[:], in0=flr[:], in1=iotaF[:], op=OP.add)

            # ---- Bernstein basis via Horner ----
            bv = []
            for k in range(DEG + 1):
                m3, m2, m1, m0 = (Mconst[3][k], Mconst[2][k],
                                  Mconst[1][k], Mconst[0][k])
                h = wp.tile([BPC, D_IN], F32, tag=f"bv{k}")
                nc.scalar.activation(h[:], t[:], AF.Copy, bias=m2, scale=m3)
                nc.vector.tensor_tensor(out=h[:], in0=h[:], in1=t[:], op=OP.mult)
                nc.scalar.activation(h[:], h[:], AF.Copy, bias=m1, scale=1.0)
                nc.vector.tensor_tensor(out=h[:], in0=h[:], in1=t[:], op=OP.mult)
                nc.scalar.activation(h[:], h[:], AF.Copy, bias=m0, scale=1.0)
                bv.append(h)

            # ---- transpose offsets and bv to [128 i, BPC b] ----
            offsT = []
            bvT = [[None] * NCH for _ in range(DEG + 1)]
            for c in range(NCH):
                sl = slice(c * 128, (c + 1) * 128)
                pt = ptr.tile([128, BPC], F32, tag="ptr")
                nc.tensor.transpose(pt[:], offsF[:, sl], ident[:BPC, :BPC])
                ot = cp.tile([128, BPC], I32, tag=f"offsT{c}")
                nc.vector.tensor_copy(ot[:], pt[:])
                offsT.append(ot)
                for k in range(DEG + 1):
                    pb = ptr.tile([128, BPC], F32, tag="ptr")
                    nc.tensor.transpose(pb[:], bv[k][:, sl], ident[:BPC, :BPC])
                    bt_ = cp.tile([128, BPC], F16, tag=f"bvT{k}_{c}")
                    nc.vector.tensor_copy(bt_[:], pb[:])
                    bvT[k][c] = bt_

            # ---- SUMBV[b] = sum_{i,k} fp16(bv) via ones-vector matmuls ----
            ones = cp.tile([128, 1], F16, tag="ones")
            nc.vector.memset(ones[:], 1.0)
            sacc = ptr.tile([1, BPC], F32, tag="ptr", name="sacc")
            n_mm = NCH * (DEG + 1)
            j = 0
            for c in range(NCH):
                for k in range(DEG + 1):
                    nc.tensor.matmul(sacc[:], lhsT=ones[:], rhs=bvT[k][c][:],
                                     start=(j == 0), stop=(j == n_mm - 1))
                    j += 1
            # CB[b] = (1151.5/8) * SUMBV[b], row vector on partition 0
            cb = cp.tile([1, BPC], F32, tag="cb")
            nc.vector.tensor_scalar_mul(cb[:], sacc[:], 1151.5 / 8.0)

            # ---- gather + unpack + contract ----
            if repeat == 0:
                zrow = op_.tile([BPC, D_OUT], F32, tag="zrow")
                nc.vector.memset(zrow[:], 0.0)
                nc.sync.dma_start(y8[:, :], zrow[:])
            for _rep in range(repeat):
                accs = [pacc.tile([1, D_OUT], F32, tag=f"acc{b}",
                                  name=f"acc{b}")
                        for b in range(BPC - 1)]
                accs.append(ptr.tile([1, D_OUT], F32, tag="ptr", name="acc7"))
                for b in range(BPC):
                    acc = accs[b]
                    for c in range(NCH):
                        G8 = g8p.tile([128, ROW], U8, tag="G8", name="G8")
                        nc.gpsimd.indirect_dma_start(
                            out=G8[:],
                            out_offset=None,
                            in_=R[:],
                            in_offset=bass.IndirectOffsetOnAxis(
                                ap=offsT[c][:, b:b + 1], axis=0),
                        )
                        Gf = gfp.tile([128, ROW], F16, tag="Gf", name="Gf")
                        x16 = G8[:].bitcast(U16)
                        nc.vector.tensor_scalar(
                            Gf[:, 0:ROW // 2].bitcast(U16), x16,
                            0x00FF, 0x5800, OP.bitwise_and, OP.bitwise_or)
                        nc.vector.tensor_scalar(
                            Gf[:, ROW // 2:ROW].bitcast(U16), x16,
                            8, 0x5800, OP.logical_shift_right, OP.bitwise_or)
                        for k in range(DEG + 1):
                            nc.tensor.matmul(
                                acc[:],
                                lhsT=bvT[k][c][:, b:b + 1],
                                rhs=Gf[:, k * D_OUT:(k + 1) * D_OUT],
                                start=(c == 0 and k == 0),
                                stop=(c == NCH - 1 and k == DEG),
                            )
                # all epilogues after the full matmul stream: the in-order
                # DVE must not stall mid-iteration behind PE-dependent work
                for b in range(BPC):
                    orow = op_.tile([1, D_OUT], F32, tag="orow")
                    nc.vector.tensor_scalar(orow[:], accs[b][:],
                                            cb[0:1, b:b + 1], 8.0 * s,
                                            OP.subtract, OP.mult)
                    nc.sync.dma_start(y8[b:b + 1, :], orow[:])

    nc.compile()
    return nc


def prep_R(poly_matrix):
    """poly[i, o, g, k] -> R[i, g, k, o] -> [NROWS, ROW] uint8, with each
    row byte-permuted so the fp16 unpack lands contiguously:
    stored[2m] = u[m], stored[2m+1] = u[1024+m].

    Returns (R_u8, s) with P ~= s * (u - 127.5)."""
    key = poly_matrix.ctypes.data if isinstance(poly_matrix, np.ndarray) else None
    if key is not None and key in _RPREP:
        return _RPREP[key]
    Rm = np.ascontiguousarray(np.transpose(np.asarray(poly_matrix, np.float32),
                                           (0, 2, 3, 1))).reshape(NROWS, ROW)
    s = float(np.abs(Rm).max()) / 127.5
    u = np.clip(np.round(Rm / s + 127.5), 0, 255).astype(np.uint8)
    perm = np.empty_like(u)
    perm[:, 0::2] = u[:, :ROW // 2]
    perm[:, 1::2] = u[:, ROW // 2:]
    out = (np.ascontiguousarray(perm), s)
    if key is not None:
        _RPREP[key] = out
    return out


def get_compiled(basis_matrix, ln_weight, ln_bias, s, repeat=1):
    apply_affine = not (np.all(ln_weight == 1.0) and np.all(ln_bias == 0.0))
    Mkey = np.asarray(basis_matrix, np.float32).tobytes()
    key = (Mkey, apply_affine, s, repeat)
    if key not in _CACHE:
        Mconst = [[float(basis_matrix[j, k]) for k in range(DEG + 1)]
                  for j in range(DEG + 1)]
        _CACHE[key] = _build_nc(Mconst, apply_affine, s, repeat)
    return _CACHE[key], apply_affine


def make_in_maps(x, R_u8, ln_weight, ln_bias, apply_affine):
    x = np.asarray(x, np.float32)
    maps = []
    for c in range(N_CORES):
        m = {"x8": np.ascontiguousarray(x[c * BPC:(c + 1) * BPC]), "R": R_u8}
        if apply_affine:
            m["w8"] = np.ascontiguousarray(
                np.broadcast_to(np.asarray(ln_weight, np.float32), (BPC, D_IN)))
            m["b8"] = np.ascontiguousarray(
                np.broadcast_to(np.asarray(ln_bias, np.float32), (BPC, D_IN)))
        maps.append(m)
    return maps


def kernel(x, poly_matrix, ln_weight, ln_bias, basis_matrix):
    R_u8, s = prep_R(np.asarray(poly_matrix))
    nc, apply_affine = get_compiled(basis_matrix, ln_weight, ln_bias, s)
    in_maps = make_in_maps(x, R_u8, ln_weight, ln_bias, apply_affine)
    res = run_bass_kernel_spmd(nc, in_maps, core_ids=list(range(N_CORES)))
    y = np.concatenate([res.results[c]["y8"] for c in range(N_CORES)], axis=0)
    return y.astype(np.float32)
